# revision 29
# baseline (speedup 1.0000x reference)
"""Distributed GAT layer kernel for 8 TRN2 NeuronCores.

Row-parallel over the 4096 query nodes; NO collective: each core redundantly
computes the full projection Wh = H @ W (same FLOP count as its attention
share), fed by a host-transposed H.T so no on-device transposes are needed.

Host prep per core k:
  - node order rotated by -512k so the core's local nodes are always chunk 0
    (makes the SPMD program core-independent: sl broadcast reads chunk 0)
  - transposed multiplicative mask ATM[j, q] in {0, 1} fp16, self-loops baked

Device structure (single fused loop, dependency-scheduled by Tile):
  per key-tile i (128 nodes):
  - every 4th iteration: one slsr chunk = slsr projection (wlr-stationary
    matmul -> [8, 512] slsr.T), PE transposes into per-key layout, per-chunk
    exp(sr)/exp(.2 sr); chunk 0 also builds the sl broadcast + its exps.
  - projection Wh tile (4 accumulating matmuls, N=512) -> WHA fp16 (+ones col)
  - scores on two balanced engine paths sharing the {0,1} mask:
    DVE path (separable exp): exp(lrelu(s)) = max(exp s, exp .2s) with
    s = sl + sr rank-1 => P = max(El*Er, el*er) * m; no ACT exp on big tiles.
    ACT path: s by TS adds, ACT Prelu + Exp, mask-mult on DVE.
  - attention matmuls (16 accumulation groups in 6 PSUM banks, [128,3,170]
    packing; accumulate-only into pre-zeroed PSUM) trail by LAG tiles.
All slsr-phase PSUM shares the 2 rotating projection slots, so accs(6)+proj(2)
fit the 8 banks and the accumulator memsets run at t=0.
"""

import sys

sys.path.insert(0, "/opt/trn_rl_repo")

import numpy as np

N = 4096
D = 512
HEADS = 4
DK = 128
NCORES = 8
CQ = N // NCORES          # query rows per core = 512
NRT = N // 128            # 32 key/row tiles
NC5 = N // 512            # 8 chunks of 512 for the slsr projection
LAG = 6                   # attention matmuls trail the projection by LAG tiles

_CACHE = {}


def _build(debug=False):
    import concourse.bass as bass
    import concourse.mybir as mybir
    from concourse import bacc, tile

    f32 = mybir.dt.float32
    fp16 = mybir.dt.float16
    AF = mybir.ActivationFunctionType
    OP = mybir.AluOpType

    nc = bacc.Bacc(
        "TRN2",
        target_bir_lowering=False,
        debug=debug,
        enable_asserts=True,
        num_devices=NCORES,
    )

    HT = nc.dram_tensor("ht", [D, N], fp16, kind="ExternalInput")
    WB = nc.dram_tensor("wb", [D, 520], fp16, kind="ExternalInput")
    ATM = nc.dram_tensor("atm", [N, CQ], fp16, kind="ExternalInput")
    IDENT = nc.dram_tensor("ident", [128, 128], fp16, kind="ExternalInput")
    SEL = nc.dram_tensor("sel", [8, HEADS, 128], fp16, kind="ExternalInput")
    OUT = nc.dram_tensor("out", [CQ, D], f32, kind="ExternalOutput")

    # jt tiles routed to the ACT path (Prelu+Exp there), rest on the DVE
    # separable path; fraction tuned to balance the two engines.
    NACT = 21
    act_path = set()
    acc_n = 0
    for jt in range(NRT):
        acc_n += NACT
        if acc_n >= NRT:
            acc_n -= NRT
            act_path.add(jt)

    with tile.TileContext(nc) as tc:
        with (
            tc.tile_pool(name="const", bufs=1) as constp,
            tc.tile_pool(name="outp", bufs=1) as outp,
            tc.tile_pool(name="sp", bufs=3) as spp,
            tc.tile_pool(name="tp", bufs=3) as tpp,
            tc.tile_pool(name="wp", bufs=3) as wpp,
            tc.tile_pool(name="pp", bufs=7) as ppp,
            tc.tile_pool(name="slp", bufs=2) as slp,
            tc.tile_pool(name="psacc", bufs=1, space="PSUM") as psaccp,
            tc.tile_pool(name="psmix", bufs=2, space="PSUM") as psmixp,
        ):
            # ---------------- DMA loads ----------------
            idb = constp.tile([128, 128], fp16, tag="idb")
            nc.sync.dma_start(idb[:], IDENT[:])
            sel = constp.tile([8, HEADS, 128], fp16, tag="sel")
            nc.sync.dma_start(sel[:], SEL[:])
            wb = constp.tile([128, 4, 520], fp16, tag="wb")
            nc.sync.dma_start(wb[:], WB.rearrange("(a p) d -> p a d", p=128))
            hb = constp.tile([128, 4, N], fp16, tag="hb")
            hre = HT.rearrange("(a p) j -> p a j", p=128)
            at = constp.tile([128, NRT, CQ], fp16, tag="at")
            are = ATM.rearrange("(jt p) q -> p jt q", p=128)
            # hb gates phase 0 + projection; at chunk 0 gates the first
            # scores; remaining at chunks trail
            order = [("hb", 0), ("at", 0)] + [("hb", c) for c in range(1, NC5)] \
                + [("at", c) for c in range(1, NC5)]
            for kind, c in order:
                if kind == "hb":
                    nc.sync.dma_start(
                        hb[:, :, c * 512:(c + 1) * 512],
                        hre[:, :, c * 512:(c + 1) * 512],
                    )
                else:
                    nc.sync.dma_start(
                        at[:, c * 4:(c + 1) * 4, :], are[:, c * 4:(c + 1) * 4, :]
                    )

            # WHA [j, jt, h, dk | ones | pad] fp16
            WHA = constp.tile([128, NRT, HEADS, DK + 2], fp16, tag="WHA")
            nc.gpsimd.memset(WHA[:, :, :, DK:DK + 1], 1.0)

            # ACT exp table preload (off the critical path)
            dumm = constp.tile([128, 8], fp16, tag="dumm")
            nc.gpsimd.memset(dumm[:], 0.0)
            nc.scalar.activation(dumm[:], dumm[:], AF.Exp)

            srslH = constp.tile([128, NRT, 8], fp16, tag="srslH")
            srsl32 = constp.tile([128, NRT, 8], f32, tag="srsl32")
            SLBC = constp.tile([128, HEADS, CQ], fp16, tag="SLBC")
            srslE = constp.tile([128, NRT, HEADS], f32, tag="srslE")
            srsle = constp.tile([128, NRT, HEADS], f32, tag="srsle")
            ElBC = constp.tile([128, HEADS, CQ], fp16, tag="ElBC")
            elBC = constp.tile([128, HEADS, CQ], fp16, tag="elBC")


            # HAM warm-up: a burst of dummy matmuls on the identity tile as
            # soon as it lands, so the PE is at 2.4 GHz when real work starts
            for wu in range(10):
                pw = psmixp.tile([128, 512], f32, tag="ps", name="pw")
                for rep in range(4):
                    nc.tensor.matmul(
                        pw[:, 0:128], idb[:], idb[:],
                        start=(rep == 0), stop=(rep == 3),
                    )

            # 16 attention accumulation groups packed 3-per-bank: group
            # g = qt*4+h lives at accs[g//3][:, g%3, :].  Pre-zeroed once;
            # matmuls accumulate-only.
            accs = [
                psaccp.tile([128, 3, 170], f32, tag=f"acc{i}", name=f"acc{i}")
                for i in range(6)
            ]
            for a in accs:
                nc.vector.memset(a[:], 0.0)

            def emit_slsr_chunk(c):
                slsrT = slp.tile([8, 512], fp16, tag="slsrT", name="slsrT")
                p0 = psmixp.tile([8, 512], f32, tag="ps", name="p0")
                for ct in range(4):
                    nc.tensor.matmul(
                        p0[:],
                        wb[:, ct, 512:520],
                        hb[:, ct, c * 512:(c + 1) * 512],
                        start=(ct == 0),
                        stop=(ct == 3),
                    )
                nc.vector.tensor_copy(slsrT[:], p0[:])
                if c == 0:
                    # SLBC: broadcast sl of the local (=first) chunk
                    for h in range(HEADS):
                        pb = psmixp.tile([128, 512], f32, tag="ps", name="pb")
                        nc.tensor.matmul(
                            pb[:], sel[:, h, :], slsrT[:],
                            start=True, stop=True,
                        )
                        nc.vector.tensor_copy(SLBC[:, h, :], pb[:])
                    nc.scalar.activation(ElBC[:], SLBC[:], AF.Exp)
                    nc.scalar.activation(elBC[:], SLBC[:], AF.Exp, scale=0.2)
                for b in range(4):
                    pt = psmixp.tile([128, 8], fp16, tag="ps", name="pt")
                    nc.tensor.transpose(
                        pt[:], slsrT[0:8, b * 128:(b + 1) * 128], idb[0:8, 0:8]
                    )
                    nc.vector.tensor_copy(srslH[:, c * 4 + b, :], pt[:])
                cc = slice(c * 4, c * 4 + 4)
                nc.vector.tensor_copy(srsl32[:, cc, 4:8], srslH[:, cc, 4:8])
                nc.scalar.activation(srslE[:, cc, :], srslH[:, cc, 4:8], AF.Exp)
                nc.scalar.activation(
                    srsle[:, cc, :], srslH[:, cc, 4:8], AF.Exp, scale=0.2
                )

            pps = []

            def emit_scores(jt):
                pp = ppp.tile([128, HEADS, CQ], fp16, tag="pp", name="pp")
                if jt in act_path:
                    # s = sl + sr, Prelu+Exp on ACT, mask-mult on DVE
                    sp = spp.tile([128, HEADS, CQ], fp16, tag="sp", name="sp")
                    for h in range(HEADS):
                        nc.vector.tensor_scalar(
                            sp[:, h, :], SLBC[:, h, :],
                            srsl32[:, jt, 4 + h:5 + h], None, op0=OP.add,
                        )
                    tt = tpp.tile([128, HEADS, CQ], fp16, tag="tt", name="tt")
                    nc.scalar.activation(tt[:], sp[:], AF.Prelu, alpha=0.2)
                    w = wpp.tile([128, HEADS, CQ], fp16, tag="w", name="w")
                    nc.scalar.activation(w[:], tt[:], AF.Exp)
                else:
                    # separable: P = max(exp(sl)exp(sr), exp(.2sl)exp(.2sr))*m
                    sp = spp.tile([128, HEADS, CQ], fp16, tag="sp", name="sp")
                    for h in range(HEADS):
                        nc.vector.tensor_scalar(
                            sp[:, h, :], ElBC[:, h, :],
                            srslE[:, jt, h:h + 1], None, op0=OP.mult,
                        )
                    tt = tpp.tile([128, HEADS, CQ], fp16, tag="tt", name="tt")
                    for h in range(HEADS):
                        nc.vector.tensor_scalar(
                            tt[:, h, :], elBC[:, h, :],
                            srsle[:, jt, h:h + 1], None, op0=OP.mult,
                        )
                    w = wpp.tile([128, HEADS, CQ], fp16, tag="w", name="w")
                    nc.vector.tensor_tensor(w[:], sp[:], tt[:], op=OP.max)
                ab, wf = bass.broadcast_tensor_aps(at[:, jt:jt + 1, :], w[:])
                nc.vector.tensor_tensor(pp[:], wf, ab, op=OP.mult)
                pps.append(pp)

            def emit_attn(jt):
                pp = pps[jt]
                for qt in range(4):
                    for h in range(HEADS):
                        g = qt * HEADS + h
                        nc.tensor.matmul(
                            accs[g // 3][:, g % 3, 0:DK + 1],
                            pp[:, h, qt * 128:(qt + 1) * 128],
                            WHA[:, jt, h, 0:DK + 1],
                            start=False,
                            stop=False,
                            skip_group_check=True,
                        )

            # ---------------- fused main loop ----------------
            for i in range(NRT):
                if i % 4 == 0:
                    emit_slsr_chunk(i // 4)
                ps = psmixp.tile([128, 512], f32, tag="ps", name="ps")
                for ct in range(4):
                    nc.tensor.matmul(
                        ps[:],
                        hb[:, ct, i * 128:(i + 1) * 128],
                        wb[:, ct, 0:512],
                        start=(ct == 0),
                        stop=(ct == 3),
                    )
                src = ps.rearrange("p (h d) -> p h d", h=HEADS)
                nc.scalar.activation(WHA[:, i, :, 0:DK], src, AF.Copy)
                emit_scores(i)
                if i >= LAG:
                    emit_attn(i - LAG)
            for j in range(NRT - LAG, NRT):
                emit_attn(j)

            # ---------------- Epilogue: 1/D scale + ELU ----------------
            # stage-major so the four qt streams pipeline across engines;
            # reciprocals batched per acc tile; ELU internals in fp16
            rec = outp.tile([128, 16], f32, tag="rec")
            for t in range(6):
                n_g = 3 if t < 5 else 1
                nc.vector.reciprocal(
                    rec[:, 3 * t:3 * t + n_g],
                    accs[t][:, 0:n_g, DK:DK + 1].rearrange("p a b -> p (a b)"),
                )
            os_, ms_, es_, rs_, ofs_ = [], [], [], [], []
            for qt in range(4):
                o = outp.tile([128, HEADS, DK], fp16, tag=f"o{qt}", name="o")
                for h in range(HEADS):
                    g = qt * HEADS + h
                    acc = accs[g // 3]
                    s = g % 3
                    if h % 2 == 0:
                        nc.vector.tensor_scalar(
                            o[:, h, :], acc[:, s, 0:DK], rec[:, g:g + 1],
                            None, op0=OP.mult,
                        )
                    else:
                        nc.scalar.activation(
                            o[:, h, :], acc[:, s, 0:DK], AF.Copy,
                            scale=rec[:, g:g + 1],
                        )
                os_.append(o)
            for qt in range(4):
                m = outp.tile([128, HEADS, DK], fp16, tag=f"m{qt}", name="m")
                nc.vector.tensor_scalar(m[:], os_[qt][:], 0.0, None, op0=OP.min)
                ms_.append(m)
            for qt in range(4):
                e = outp.tile([128, HEADS, DK], fp16, tag=f"e{qt}", name="e")
                nc.scalar.activation(e[:], ms_[qt][:], AF.Exp)
                es_.append(e)
            for qt in range(4):
                r = outp.tile([128, HEADS, DK], fp16, tag=f"r{qt}", name="r")
                nc.vector.tensor_scalar(r[:], os_[qt][:], 0.0, None, op0=OP.max)
                rs_.append(r)
            for qt in range(4):
                of = outp.tile([128, HEADS, DK], f32, tag=f"of{qt}", name="of")
                nc.vector.scalar_tensor_tensor(
                    of[:], es_[qt][:], 1.0, rs_[qt][:],
                    op0=OP.subtract, op1=OP.add,
                )
                ofs_.append(of)
            for qt in range(4):
                nc.sync.dma_start(OUT[qt * 128:(qt + 1) * 128, :], ofs_[qt][:])

    return nc


def _prep_inputs(H, A, W, a_l, a_r):
    Wf = np.asarray(W, dtype=np.float32)
    wl = np.einsum("chd,hd->ch", Wf.reshape(D, HEADS, DK), a_l).astype(np.float32)
    wr = np.einsum("chd,hd->ch", Wf.reshape(D, HEADS, DK), a_r).astype(np.float32)
    wb = np.ascontiguousarray(
        np.concatenate([Wf, wl, wr], axis=1)
    ).astype(np.float16)
    ident = np.eye(128, dtype=np.float16)
    sel = np.zeros((8, HEADS, 128), dtype=np.float16)
    for h in range(HEADS):
        sel[h, h, :] = 1.0

    M = A > 0
    idx = np.arange(N)
    M[idx, idx] = True
    HTf = H.T.astype(np.float16)  # [512, 4096]

    in_maps = []
    for k in range(NCORES):
        rot = np.roll(np.arange(N), -CQ * k)  # local nodes first
        HTk = np.ascontiguousarray(HTf[:, rot])
        ATMk = np.where(
            M[CQ * k:CQ * (k + 1), :].T[rot, :], np.float16(1.0), np.float16(0.0)
        )
        in_maps.append(
            {
                "ht": HTk,
                "wb": wb,
                "atm": np.ascontiguousarray(ATMk),
                "ident": ident,
                "sel": sel,
            }
        )
    return in_maps


def kernel(H, A, W, a_l, a_r, _trace=False):
    from concourse.bass_utils import run_bass_kernel_spmd

    H = np.asarray(H, dtype=np.float32)
    A = np.asarray(A, dtype=np.int32)
    W = np.asarray(W, dtype=np.float32)
    a_l = np.asarray(a_l, dtype=np.float32)
    a_r = np.asarray(a_r, dtype=np.float32)

    if "nc" not in _CACHE:
        nc = _build()
        nc.finalize()  # Bacc register allocation; required for the PJRT path
        _CACHE["nc"] = nc
    nc = _CACHE["nc"]

    in_maps = _prep_inputs(H, A, W, a_l, a_r)
    kw = {}
    if _trace:
        import tempfile

        kw["tmpdir"] = tempfile.mkdtemp(prefix="gat_trace_")
        _CACHE["tmpdir"] = kw["tmpdir"]
    res = run_bass_kernel_spmd(
        nc, in_maps, core_ids=list(range(NCORES)), trace=_trace, **kw
    )
    out = np.concatenate([res.results[k]["out"] for k in range(NCORES)], axis=0)
    if _trace:
        _CACHE["exec_time_ns"] = res.exec_time_ns
        _CACHE["profile_json"] = res.profile_json
    return out
